# revision 16
# baseline (speedup 1.0000x reference)
"""MoE AdaptiveExpertLayer on 8 TRN2 NeuronCores (expert-parallel Bass kernel).

Sharding (hardcoded): expert-parallel — core e owns expert e's W1/b1/W2/b2.
The router (gate matmul + softmax + top-2, ~0.01% of total FLOPs) runs on the
host during input sharding; tokens are dispatched to their two chosen experts'
cores as capacity-padded batches ("all-to-all dispatch by router choice" done
at the sharding step).  Each core runs the expert MLP
    y = (relu(x @ W1.T + b1) @ W2.T + b2) * combine_weight
over its C dispatched tokens, in bf16 with fp32 PSUM accumulation, weights
fully SBUF-resident.  The host sums each token's two expert contributions.

Problem shapes: x [4, 2048, 1024], W1 [8, 4096, 1024], W2 [8, 1024, 4096].
"""

import time

import numpy as np
import ml_dtypes
from contextlib import ExitStack

import concourse.tile as tile
from concourse import bacc, mybir
from concourse.tile import add_dep_helper
from concourse.bass_utils import run_bass_kernel_spmd

D_MODEL = 1024
D_FF = 4096
N_EXPERTS = 8
TOP_K = 2
N_CORES = 8
CAPACITY = 2176  # default per-expert token capacity (multiple of 128)

BF16 = mybir.dt.bfloat16
F32 = mybir.dt.float32
_BF = ml_dtypes.bfloat16

# Set by callers that want NTFF profiling; BASS_TRACE=1 env also works.
TRACE = False
LAST_RESULTS = None

_graph_cache = {}


def _token_blocks(c):
    """Split capacity into matmul token-blocks of <=512 (multiples of 128)."""
    blocks = []
    t0 = 0
    while t0 < c:
        tb = min(512, c - t0)
        blocks.append((t0, tb))
        t0 += tb
    return blocks


def _build_graph(c):
    """Build + compile the per-core expert-MLP Bass graph for capacity c."""
    nc = bacc.Bacc("TRN2", target_bir_lowering=False, debug=False,
                   num_devices=N_CORES)

    xt = nc.dram_tensor("xt", [D_MODEL, c], BF16, kind="ExternalInput").ap()
    w1t = nc.dram_tensor("w1t", [D_MODEL, D_FF], BF16, kind="ExternalInput").ap()
    w2t = nc.dram_tensor("w2t", [D_FF, D_MODEL], BF16, kind="ExternalInput").ap()
    b1 = nc.dram_tensor("b1", [128, D_FF // 128], F32, kind="ExternalInput").ap()
    b2bc = nc.dram_tensor("b2bc", [128, D_MODEL], F32, kind="ExternalInput").ap()
    s = nc.dram_tensor("s", [128, c // 128], F32, kind="ExternalInput").ap()
    out = nc.dram_tensor("out", [c, D_MODEL], F32, kind="ExternalOutput").ap()

    n_k1 = D_MODEL // 128   # 8  contraction chunks for matmul 1
    n_m1 = D_FF // 128      # 32 output tiles for matmul 1
    n_dn = D_MODEL // 512   # 2  output column tiles for matmul 2

    with tile.TileContext(nc) as tc, ExitStack() as ctx:
        wp1 = ctx.enter_context(tc.tile_pool(name="w1", bufs=n_k1))
        wp2 = ctx.enter_context(tc.tile_pool(name="w2", bufs=n_m1))
        cpool = ctx.enter_context(tc.tile_pool(name="consts", bufs=2))
        b2pool = ctx.enter_context(tc.tile_pool(name="b2p", bufs=1))
        xpool = ctx.enter_context(tc.tile_pool(name="x", bufs=n_k1))
        hpool = ctx.enter_context(tc.tile_pool(name="h", bufs=n_m1))
        opool = ctx.enter_context(tc.tile_pool(name="o", bufs=4))
        pp1 = ctx.enter_context(tc.tile_pool(name="p1", bufs=5, space="PSUM"))
        pp2 = ctx.enter_context(tc.tile_pool(name="p2", bufs=3, space="PSUM"))

        # Block-0 x tiles + biases land first so the PE can start ASAP; w1 is
        # loaded in progressively larger column segments across DMA queues,
        # issued from the (otherwise idle) vector queue.
        blocks = _token_blocks(c)
        t0_0, tb_0 = blocks[0]
        b1_all = cpool.tile([128, n_m1], F32, tag="b1a")
        nc.sync.dma_start(b1_all[:], b1[:, :])
        b1_tiles = [b1_all[:, m:m + 1] for m in range(n_m1)]

        x0_tiles = []
        for k in range(n_k1):
            t = xpool.tile([128, tb_0], BF16, tag="x")
            nc.scalar.dma_start(t[:], xt[k * 128:(k + 1) * 128, t0_0:t0_0 + tb_0])
            x0_tiles.append(t)

        w1_tiles = [wp1.tile([128, D_FF], BF16, tag="w1", name=f"w1c{k}")
                    for k in range(n_k1)]
        seg_bounds = [0, 512, 1024, 1536, 2048, 2560, 3072, 3584, D_FF]
        for lo, hi in zip(seg_bounds[:-1], seg_bounds[1:]):
            for k in range(n_k1):
                nc.sync.dma_start(
                    w1_tiles[k][:, lo:hi],
                    w1t[k * 128:(k + 1) * 128, lo:hi])

        w2_tiles = [wp2.tile([128, D_MODEL], BF16, tag="w2", name=f"w2c{k}")
                    for k in range(n_m1)]
        s_all = cpool.tile([128, c // 128], F32, tag="sa")
        b2_tile = b2pool.tile([128, D_MODEL], F32, tag="b2")

        first = True
        for (t0, tb) in blocks:
            if first:
                x_tiles = x0_tiles
            else:
                x_tiles = []
                for k in range(n_k1):
                    t = xpool.tile([128, tb], BF16, tag="x")
                    nc.sync.dma_start(t[:],
                                      xt[k * 128:(k + 1) * 128, t0:t0 + tb])
                    x_tiles.append(t)

            # h.T [D_FF, tb] = relu(W1 @ x.T + b1), FF on partitions
            h_tiles = []
            relu_insts = []
            for m in range(n_m1):
                ps = pp1.tile([128, tb], F32, tag="p1")
                for k in range(n_k1):
                    nc.tensor.matmul(
                        ps[:], lhsT=w1_tiles[k][:, m * 128:(m + 1) * 128],
                        rhs=x_tiles[k][:], start=(k == 0), stop=(k == n_k1 - 1))
                h = hpool.tile([128, tb], BF16, tag="h")
                ri = nc.scalar.activation(h[:], ps[:],
                                          mybir.ActivationFunctionType.Relu,
                                          bias=b1_tiles[m][:])
                relu_insts.append(ri)
                h_tiles.append(h)

            if first:
                # w2 / s / b2 only gate matmul 2 — load them behind m1 on the
                # gpsimd queue, held back until block-0 m1 is underway so the
                # w1 segment loads (which the PE is waiting on) get the HBM
                # bandwidth first.
                first = False
                for k in range(n_m1):
                    d = nc.gpsimd.dma_start(w2_tiles[k][:],
                                            w2t[k * 128:(k + 1) * 128, :])
                    add_dep_helper(d.ins, relu_insts[10].ins, sync=True,
                                   reason="w2 load behind early m1")
                nc.gpsimd.dma_start(s_all[:], s[:, :])
                nc.gpsimd.dma_start(b2_tile[:], b2bc[:, :])

            # y [tb, D_MODEL] = (h @ W2.T + b2) * s, tokens on partitions
            for tm in range(tb // 128):
                g = (t0 + tm * 128) // 128
                for dn in range(n_dn):
                    ps = pp2.tile([128, 512], F32, tag="p2")
                    for k in range(n_m1):
                        nc.tensor.matmul(
                            ps[:], lhsT=h_tiles[k][:, tm * 128:(tm + 1) * 128],
                            rhs=w2_tiles[k][:, dn * 512:(dn + 1) * 512],
                            start=(k == 0), stop=(k == n_m1 - 1))
                    t = opool.tile([128, 512], F32, tag="t")
                    nc.vector.tensor_add(t[:], ps[:],
                                         b2_tile[:, dn * 512:(dn + 1) * 512])
                    o = opool.tile([128, 512], F32, tag="o")
                    nc.scalar.mul(o[:], t[:], s_all[:, g:g + 1])
                    nc.sync.dma_start(
                        out[t0 + tm * 128:t0 + (tm + 1) * 128,
                            dn * 512:(dn + 1) * 512], o[:])

    nc.compile()
    return nc


def _get_graph(c):
    if c not in _graph_cache:
        _graph_cache[c] = _build_graph(c)
    return _graph_cache[c]


def kernel(x, gate_w, W1, b1, W2, b2):
    global LAST_RESULTS
    xt2 = np.ascontiguousarray(x.reshape(-1, D_MODEL)).astype(np.float32)
    n = xt2.shape[0]

    # --- host router (tiny: [N,1024]@[1024,8]) ---
    logits = xt2 @ gate_w.astype(np.float32).T
    logits -= logits.max(axis=-1, keepdims=True)
    probs = np.exp(logits)
    probs /= probs.sum(axis=-1, keepdims=True)
    top2 = np.argsort(-probs, axis=-1, kind="stable")[:, :TOP_K]
    wt = np.take_along_axis(probs, top2, axis=-1)
    wt = wt / (wt.sum(axis=-1, keepdims=True) + 1e-9)

    # --- dispatch: sort (token, expert) pairs by expert ---
    flat_e = top2.ravel()
    flat_t = np.repeat(np.arange(n), TOP_K)
    flat_w = wt.ravel()
    order = np.argsort(flat_e, kind="stable")
    e_sorted = flat_e[order]
    t_sorted = flat_t[order]
    w_sorted = flat_w[order]
    counts = np.bincount(e_sorted, minlength=N_EXPERTS)
    starts = np.zeros(N_EXPERTS + 1, dtype=np.int64)
    starts[1:] = np.cumsum(counts)

    c = max(CAPACITY, int(-(-counts.max() // 128)) * 128)
    # slot of each sorted pair in the concatenated [E*C] dispatch buffer,
    # then invert to per-token pair slots for the final combine
    slot = np.arange(TOP_K * n) - starts[e_sorted] + e_sorted * c
    pair_slot = np.empty(TOP_K * n, dtype=np.int64)
    pair_slot[order] = slot
    pair_slot = pair_slot.reshape(n, TOP_K)

    w1f = np.asarray(W1, dtype=np.float32)
    w2f = np.asarray(W2, dtype=np.float32)
    b1f = np.asarray(b1, dtype=np.float32)
    b2f = np.asarray(b2, dtype=np.float32)

    in_maps = []
    for e in range(N_EXPERTS):
        sel = t_sorted[starts[e]:starts[e + 1]]
        ne = len(sel)
        xe = np.zeros((D_MODEL, c), dtype=_BF)
        xe[:, :ne] = xt2[sel].T.astype(_BF)
        se = np.zeros(c, dtype=np.float32)
        se[:ne] = w_sorted[starts[e]:starts[e + 1]]
        se = np.ascontiguousarray(se.reshape(c // 128, 128).T)
        in_maps.append({
            "xt": xe,
            "w1t": np.ascontiguousarray(w1f[e].T).astype(_BF),
            "w2t": np.ascontiguousarray(w2f[e].T).astype(_BF),
            "b1": np.ascontiguousarray(b1f[e].reshape(D_FF // 128, 128).T),
            "b2bc": np.ascontiguousarray(
                np.broadcast_to(b2f[e], (128, D_MODEL))),
            "s": se,
        })

    nc = _get_graph(c)
    res = None
    for attempt in range(4):
        try:
            res = run_bass_kernel_spmd(nc, in_maps,
                                       core_ids=list(range(N_CORES)),
                                       trace=TRACE and attempt < 3)
            break
        except Exception:
            # Transient device failures (NRT_EXEC_UNIT_UNRECOVERABLE, axon
            # profile-start) clear after the terminal resets; back off and
            # retry, dropping the profiling request on the last attempt.
            if attempt == 3:
                raise
            time.sleep(20 * (attempt + 1))
    LAST_RESULTS = res

    y_all = np.concatenate([res.results[e]["out"] for e in range(N_EXPERTS)],
                           axis=0)
    combined = y_all[pair_slot[:, 0]] + y_all[pair_slot[:, 1]]
    return combined.reshape(x.shape).astype(np.float32)


# revision 17
# speedup vs baseline: 1.0022x; 1.0022x over previous
"""MoE AdaptiveExpertLayer on 8 TRN2 NeuronCores (expert-parallel Bass kernel).

Sharding (hardcoded): expert-parallel — core e owns expert e's W1/b1/W2/b2.
The router (gate matmul + softmax + top-2, ~0.01% of total FLOPs) runs on the
host during input sharding; tokens are dispatched to their two chosen experts'
cores as capacity-padded batches ("all-to-all dispatch by router choice" done
at the sharding step).  Each core runs the expert MLP
    y = (relu(x @ W1.T + b1) @ W2.T + b2) * combine_weight
over its C dispatched tokens, in bf16 with fp32 PSUM accumulation, weights
fully SBUF-resident.  The host sums each token's two expert contributions.

Problem shapes: x [4, 2048, 1024], W1 [8, 4096, 1024], W2 [8, 1024, 4096].
"""

import time

import numpy as np
import ml_dtypes
from contextlib import ExitStack

import concourse.tile as tile
from concourse import bacc, mybir
from concourse.tile import add_dep_helper
from concourse.bass_utils import run_bass_kernel_spmd

D_MODEL = 1024
D_FF = 4096
N_EXPERTS = 8
TOP_K = 2
N_CORES = 8
CAPACITY = 2176  # default per-expert token capacity (multiple of 128)

BF16 = mybir.dt.bfloat16
F32 = mybir.dt.float32
_BF = ml_dtypes.bfloat16

# Set by callers that want NTFF profiling; BASS_TRACE=1 env also works.
TRACE = False
LAST_RESULTS = None

_graph_cache = {}


def _token_blocks(c):
    """Split capacity into matmul token-blocks of <=512 (multiples of 128)."""
    blocks = []
    t0 = 0
    while t0 < c:
        tb = min(512, c - t0)
        blocks.append((t0, tb))
        t0 += tb
    return blocks


def _build_graph(c):
    """Build + compile the per-core expert-MLP Bass graph for capacity c."""
    nc = bacc.Bacc("TRN2", target_bir_lowering=False, debug=False,
                   num_devices=N_CORES)

    xt = nc.dram_tensor("xt", [D_MODEL, c], BF16, kind="ExternalInput").ap()
    w1t = nc.dram_tensor("w1t", [D_MODEL, D_FF], BF16, kind="ExternalInput").ap()
    w2t = nc.dram_tensor("w2t", [D_FF, D_MODEL], BF16, kind="ExternalInput").ap()
    b1 = nc.dram_tensor("b1", [128, D_FF // 128], F32, kind="ExternalInput").ap()
    b2bc = nc.dram_tensor("b2bc", [128, D_MODEL], F32, kind="ExternalInput").ap()
    s = nc.dram_tensor("s", [128, c // 128], F32, kind="ExternalInput").ap()
    out = nc.dram_tensor("out", [c, D_MODEL], F32, kind="ExternalOutput").ap()

    n_k1 = D_MODEL // 128   # 8  contraction chunks for matmul 1
    n_m1 = D_FF // 128      # 32 output tiles for matmul 1
    n_dn = D_MODEL // 512   # 2  output column tiles for matmul 2

    with tile.TileContext(nc) as tc, ExitStack() as ctx:
        wp1 = ctx.enter_context(tc.tile_pool(name="w1", bufs=n_k1))
        wp2 = ctx.enter_context(tc.tile_pool(name="w2", bufs=n_m1))
        cpool = ctx.enter_context(tc.tile_pool(name="consts", bufs=2))
        b2pool = ctx.enter_context(tc.tile_pool(name="b2p", bufs=1))
        xpool = ctx.enter_context(tc.tile_pool(name="x", bufs=n_k1))
        hpool = ctx.enter_context(tc.tile_pool(name="h", bufs=n_m1))
        opool = ctx.enter_context(tc.tile_pool(name="o", bufs=4))
        pp1 = ctx.enter_context(tc.tile_pool(name="p1", bufs=5, space="PSUM"))
        pp2 = ctx.enter_context(tc.tile_pool(name="p2", bufs=3, space="PSUM"))

        # Block-0 x tiles + biases land first so the PE can start ASAP; w1 is
        # loaded in progressively larger column segments across DMA queues,
        # issued from the (otherwise idle) vector queue.
        blocks = _token_blocks(c)
        t0_0, tb_0 = blocks[0]
        b1_all = cpool.tile([128, n_m1], F32, tag="b1a")
        nc.sync.dma_start(b1_all[:], b1[:, :])
        b1_tiles = [b1_all[:, m:m + 1] for m in range(n_m1)]

        x0_tiles = []
        for k in range(n_k1):
            t = xpool.tile([128, tb_0], BF16, tag="x")
            nc.scalar.dma_start(t[:], xt[k * 128:(k + 1) * 128, t0_0:t0_0 + tb_0])
            x0_tiles.append(t)

        w1_tiles = [wp1.tile([128, D_FF], BF16, tag="w1", name=f"w1c{k}")
                    for k in range(n_k1)]
        seg_bounds = [0, 512, 1024, 1536, 2048, 2560, 3072, 3584, D_FF]
        for lo, hi in zip(seg_bounds[:-1], seg_bounds[1:]):
            for k in range(n_k1):
                nc.sync.dma_start(
                    w1_tiles[k][:, lo:hi],
                    w1t[k * 128:(k + 1) * 128, lo:hi])

        w2_tiles = [wp2.tile([128, D_MODEL], BF16, tag="w2", name=f"w2c{k}")
                    for k in range(n_m1)]
        s_all = cpool.tile([128, c // 128], F32, tag="sa")
        b2_tile = b2pool.tile([128, D_MODEL], F32, tag="b2")

        first = True
        for (t0, tb) in blocks:
            if first:
                x_tiles = x0_tiles
            else:
                x_tiles = []
                for k in range(n_k1):
                    t = xpool.tile([128, tb], BF16, tag="x")
                    nc.sync.dma_start(t[:],
                                      xt[k * 128:(k + 1) * 128, t0:t0 + tb])
                    x_tiles.append(t)

            # h.T [D_FF, tb] = relu(W1 @ x.T + b1), FF on partitions
            h_tiles = []
            relu_insts = []
            for m in range(n_m1):
                ps = pp1.tile([128, tb], F32, tag="p1")
                for k in range(n_k1):
                    nc.tensor.matmul(
                        ps[:], lhsT=w1_tiles[k][:, m * 128:(m + 1) * 128],
                        rhs=x_tiles[k][:], start=(k == 0), stop=(k == n_k1 - 1))
                h = hpool.tile([128, tb], BF16, tag="h")
                ri = nc.scalar.activation(h[:], ps[:],
                                          mybir.ActivationFunctionType.Relu,
                                          bias=b1_tiles[m][:])
                relu_insts.append(ri)
                h_tiles.append(h)

            if first:
                # w2 / s / b2 only gate matmul 2 — load them behind m1 on the
                # gpsimd queue, held back until block-0 m1 is underway so the
                # w1 segment loads (which the PE is waiting on) get the HBM
                # bandwidth first.
                first = False
                for k in range(n_m1):
                    d = nc.gpsimd.dma_start(w2_tiles[k][:],
                                            w2t[k * 128:(k + 1) * 128, :])
                    add_dep_helper(d.ins, relu_insts[4].ins, sync=True,
                                   reason="w2 load behind early m1")
                nc.gpsimd.dma_start(s_all[:], s[:, :])
                nc.gpsimd.dma_start(b2_tile[:], b2bc[:, :])

            # y [tb, D_MODEL] = (h @ W2.T + b2) * s, tokens on partitions
            for tm in range(tb // 128):
                g = (t0 + tm * 128) // 128
                for dn in range(n_dn):
                    ps = pp2.tile([128, 512], F32, tag="p2")
                    for k in range(n_m1):
                        nc.tensor.matmul(
                            ps[:], lhsT=h_tiles[k][:, tm * 128:(tm + 1) * 128],
                            rhs=w2_tiles[k][:, dn * 512:(dn + 1) * 512],
                            start=(k == 0), stop=(k == n_m1 - 1))
                    t = opool.tile([128, 512], F32, tag="t")
                    nc.vector.tensor_add(t[:], ps[:],
                                         b2_tile[:, dn * 512:(dn + 1) * 512])
                    o = opool.tile([128, 512], F32, tag="o")
                    nc.scalar.mul(o[:], t[:], s_all[:, g:g + 1])
                    nc.sync.dma_start(
                        out[t0 + tm * 128:t0 + (tm + 1) * 128,
                            dn * 512:(dn + 1) * 512], o[:])

    nc.compile()
    return nc


def _get_graph(c):
    if c not in _graph_cache:
        _graph_cache[c] = _build_graph(c)
    return _graph_cache[c]


def kernel(x, gate_w, W1, b1, W2, b2):
    global LAST_RESULTS
    xt2 = np.ascontiguousarray(x.reshape(-1, D_MODEL)).astype(np.float32)
    n = xt2.shape[0]

    # --- host router (tiny: [N,1024]@[1024,8]) ---
    logits = xt2 @ gate_w.astype(np.float32).T
    logits -= logits.max(axis=-1, keepdims=True)
    probs = np.exp(logits)
    probs /= probs.sum(axis=-1, keepdims=True)
    top2 = np.argsort(-probs, axis=-1, kind="stable")[:, :TOP_K]
    wt = np.take_along_axis(probs, top2, axis=-1)
    wt = wt / (wt.sum(axis=-1, keepdims=True) + 1e-9)

    # --- dispatch: sort (token, expert) pairs by expert ---
    flat_e = top2.ravel()
    flat_t = np.repeat(np.arange(n), TOP_K)
    flat_w = wt.ravel()
    order = np.argsort(flat_e, kind="stable")
    e_sorted = flat_e[order]
    t_sorted = flat_t[order]
    w_sorted = flat_w[order]
    counts = np.bincount(e_sorted, minlength=N_EXPERTS)
    starts = np.zeros(N_EXPERTS + 1, dtype=np.int64)
    starts[1:] = np.cumsum(counts)

    c = max(CAPACITY, int(-(-counts.max() // 128)) * 128)
    # slot of each sorted pair in the concatenated [E*C] dispatch buffer,
    # then invert to per-token pair slots for the final combine
    slot = np.arange(TOP_K * n) - starts[e_sorted] + e_sorted * c
    pair_slot = np.empty(TOP_K * n, dtype=np.int64)
    pair_slot[order] = slot
    pair_slot = pair_slot.reshape(n, TOP_K)

    w1f = np.asarray(W1, dtype=np.float32)
    w2f = np.asarray(W2, dtype=np.float32)
    b1f = np.asarray(b1, dtype=np.float32)
    b2f = np.asarray(b2, dtype=np.float32)

    in_maps = []
    for e in range(N_EXPERTS):
        sel = t_sorted[starts[e]:starts[e + 1]]
        ne = len(sel)
        xe = np.zeros((D_MODEL, c), dtype=_BF)
        xe[:, :ne] = xt2[sel].T.astype(_BF)
        se = np.zeros(c, dtype=np.float32)
        se[:ne] = w_sorted[starts[e]:starts[e + 1]]
        se = np.ascontiguousarray(se.reshape(c // 128, 128).T)
        in_maps.append({
            "xt": xe,
            "w1t": np.ascontiguousarray(w1f[e].T).astype(_BF),
            "w2t": np.ascontiguousarray(w2f[e].T).astype(_BF),
            "b1": np.ascontiguousarray(b1f[e].reshape(D_FF // 128, 128).T),
            "b2bc": np.ascontiguousarray(
                np.broadcast_to(b2f[e], (128, D_MODEL))),
            "s": se,
        })

    nc = _get_graph(c)
    res = None
    for attempt in range(4):
        try:
            res = run_bass_kernel_spmd(nc, in_maps,
                                       core_ids=list(range(N_CORES)),
                                       trace=TRACE and attempt < 3)
            break
        except Exception:
            # Transient device failures (NRT_EXEC_UNIT_UNRECOVERABLE, axon
            # profile-start) clear after the terminal resets; back off and
            # retry, dropping the profiling request on the last attempt.
            if attempt == 3:
                raise
            time.sleep(20 * (attempt + 1))
    LAST_RESULTS = res

    y_all = np.concatenate([res.results[e]["out"] for e in range(N_EXPERTS)],
                           axis=0)
    combined = y_all[pair_slot[:, 0]] + y_all[pair_slot[:, 1]]
    return combined.reshape(x.shape).astype(np.float32)


# revision 18
# speedup vs baseline: 1.0170x; 1.0148x over previous
"""MoE AdaptiveExpertLayer on 8 TRN2 NeuronCores (expert-parallel Bass kernel).

Sharding (hardcoded): expert-parallel — core e owns expert e's W1/b1/W2/b2.
The router (gate matmul + softmax + top-2, ~0.01% of total FLOPs) runs on the
host during input sharding; tokens are dispatched to their two chosen experts'
cores as capacity-padded batches ("all-to-all dispatch by router choice" done
at the sharding step).  Each core runs the expert MLP
    y = (relu(x @ W1.T + b1) @ W2.T + b2) * combine_weight
over its C dispatched tokens, in bf16 with fp32 PSUM accumulation, weights
fully SBUF-resident.  The host sums each token's two expert contributions.

Problem shapes: x [4, 2048, 1024], W1 [8, 4096, 1024], W2 [8, 1024, 4096].
"""

import time

import numpy as np
import ml_dtypes
from contextlib import ExitStack

import concourse.tile as tile
from concourse import bacc, mybir
from concourse.tile import add_dep_helper
from concourse.bass_utils import run_bass_kernel_spmd

D_MODEL = 1024
D_FF = 4096
N_EXPERTS = 8
TOP_K = 2
N_CORES = 8
CAPACITY = 2176  # default per-expert token capacity (multiple of 128)

BF16 = mybir.dt.bfloat16
F32 = mybir.dt.float32
_BF = ml_dtypes.bfloat16

# Set by callers that want NTFF profiling; BASS_TRACE=1 env also works.
TRACE = False
LAST_RESULTS = None

_graph_cache = {}


def _token_blocks(c):
    """Split capacity into matmul token-blocks of <=512 (multiples of 128)."""
    blocks = []
    t0 = 0
    while t0 < c:
        tb = min(512, c - t0)
        blocks.append((t0, tb))
        t0 += tb
    return blocks


def _build_graph(c):
    """Build + compile the per-core expert-MLP Bass graph for capacity c."""
    nc = bacc.Bacc("TRN2", target_bir_lowering=False, debug=False,
                   num_devices=N_CORES)

    xt = nc.dram_tensor("xt", [D_MODEL, c], BF16, kind="ExternalInput").ap()
    w1t = nc.dram_tensor("w1t", [D_MODEL, D_FF], BF16, kind="ExternalInput").ap()
    w2t = nc.dram_tensor("w2t", [D_FF, D_MODEL], BF16, kind="ExternalInput").ap()
    b1 = nc.dram_tensor("b1", [128, D_FF // 128], F32, kind="ExternalInput").ap()
    b2bc = nc.dram_tensor("b2bc", [128, D_MODEL], F32, kind="ExternalInput").ap()
    s = nc.dram_tensor("s", [128, c // 128], F32, kind="ExternalInput").ap()
    out = nc.dram_tensor("out", [c, D_MODEL], F32, kind="ExternalOutput").ap()

    n_k1 = D_MODEL // 128   # 8  contraction chunks for matmul 1
    n_m1 = D_FF // 128      # 32 output tiles for matmul 1
    n_dn = D_MODEL // 512   # 2  output column tiles for matmul 2

    with tile.TileContext(nc) as tc, ExitStack() as ctx:
        wp1 = ctx.enter_context(tc.tile_pool(name="w1", bufs=n_k1))
        wp2 = ctx.enter_context(tc.tile_pool(name="w2", bufs=n_m1))
        cpool = ctx.enter_context(tc.tile_pool(name="consts", bufs=2))
        b2pool = ctx.enter_context(tc.tile_pool(name="b2p", bufs=1))
        xpool = ctx.enter_context(tc.tile_pool(name="x", bufs=n_k1))
        hpool = ctx.enter_context(tc.tile_pool(name="h", bufs=n_m1))
        opool = ctx.enter_context(tc.tile_pool(name="o", bufs=4))
        pp1 = ctx.enter_context(tc.tile_pool(name="p1", bufs=5, space="PSUM"))
        pp2 = ctx.enter_context(tc.tile_pool(name="p2", bufs=3, space="PSUM"))

        # Block-0 x tiles + biases land first so the PE can start ASAP; w1 is
        # loaded in progressively larger column segments across DMA queues,
        # issued from the (otherwise idle) vector queue.
        blocks = _token_blocks(c)
        t0_0, tb_0 = blocks[0]
        b1_all = cpool.tile([128, n_m1], F32, tag="b1a")
        nc.gpsimd.dma_start(b1_all[:], b1[:, :])
        b1_tiles = [b1_all[:, m:m + 1] for m in range(n_m1)]

        x0_tiles = []
        for k in range(n_k1):
            t = xpool.tile([128, tb_0], BF16, tag="x")
            nc.scalar.dma_start(t[:], xt[k * 128:(k + 1) * 128, t0_0:t0_0 + tb_0])
            x0_tiles.append(t)

        w1_tiles = [wp1.tile([128, D_FF], BF16, tag="w1", name=f"w1c{k}")
                    for k in range(n_k1)]
        seg_bounds = [0, 512, 1024, 1536, 2048, 2560, 3072, 3584, D_FF]
        for si, (lo, hi) in enumerate(zip(seg_bounds[:-1], seg_bounds[1:])):
            eng = nc.sync if si % 2 == 0 else nc.gpsimd
            for k in range(n_k1):
                eng.dma_start(
                    w1_tiles[k][:, lo:hi],
                    w1t[k * 128:(k + 1) * 128, lo:hi])

        w2_tiles = [wp2.tile([128, D_MODEL], BF16, tag="w2", name=f"w2c{k}")
                    for k in range(n_m1)]
        s_all = cpool.tile([128, c // 128], F32, tag="sa")
        b2_tile = b2pool.tile([128, D_MODEL], F32, tag="b2")

        first = True
        for (t0, tb) in blocks:
            if first:
                x_tiles = x0_tiles
            else:
                x_tiles = []
                for k in range(n_k1):
                    t = xpool.tile([128, tb], BF16, tag="x")
                    nc.sync.dma_start(t[:],
                                      xt[k * 128:(k + 1) * 128, t0:t0 + tb])
                    x_tiles.append(t)

            # h.T [D_FF, tb] = relu(W1 @ x.T + b1), FF on partitions
            h_tiles = []
            relu_insts = []
            for m in range(n_m1):
                ps = pp1.tile([128, tb], F32, tag="p1")
                for k in range(n_k1):
                    nc.tensor.matmul(
                        ps[:], lhsT=w1_tiles[k][:, m * 128:(m + 1) * 128],
                        rhs=x_tiles[k][:], start=(k == 0), stop=(k == n_k1 - 1))
                h = hpool.tile([128, tb], BF16, tag="h")
                ri = nc.scalar.activation(h[:], ps[:],
                                          mybir.ActivationFunctionType.Relu,
                                          bias=b1_tiles[m][:])
                relu_insts.append(ri)
                h_tiles.append(h)

            if first:
                # w2 / s / b2 only gate matmul 2 — load them behind m1 on the
                # gpsimd queue, held back until block-0 m1 is underway so the
                # w1 segment loads (which the PE is waiting on) get the HBM
                # bandwidth first.
                first = False
                for k in range(n_m1):
                    d = nc.gpsimd.dma_start(w2_tiles[k][:],
                                            w2t[k * 128:(k + 1) * 128, :])
                    add_dep_helper(d.ins, relu_insts[4].ins, sync=True,
                                   reason="w2 load behind early m1")
                nc.gpsimd.dma_start(s_all[:], s[:, :])
                nc.gpsimd.dma_start(b2_tile[:], b2bc[:, :])

            # y [tb, D_MODEL] = (h @ W2.T + b2) * s, tokens on partitions
            for tm in range(tb // 128):
                g = (t0 + tm * 128) // 128
                for dn in range(n_dn):
                    ps = pp2.tile([128, 512], F32, tag="p2")
                    for k in range(n_m1):
                        nc.tensor.matmul(
                            ps[:], lhsT=h_tiles[k][:, tm * 128:(tm + 1) * 128],
                            rhs=w2_tiles[k][:, dn * 512:(dn + 1) * 512],
                            start=(k == 0), stop=(k == n_m1 - 1))
                    t = opool.tile([128, 512], F32, tag="t")
                    nc.vector.tensor_add(t[:], ps[:],
                                         b2_tile[:, dn * 512:(dn + 1) * 512])
                    o = opool.tile([128, 512], F32, tag="o")
                    nc.scalar.mul(o[:], t[:], s_all[:, g:g + 1])
                    nc.sync.dma_start(
                        out[t0 + tm * 128:t0 + (tm + 1) * 128,
                            dn * 512:dn * 512 + 256], o[:, 0:256])
                    nc.sync.dma_start(
                        out[t0 + tm * 128:t0 + (tm + 1) * 128,
                            dn * 512 + 256:(dn + 1) * 512], o[:, 256:512])

    nc.compile()
    return nc


def _get_graph(c):
    if c not in _graph_cache:
        _graph_cache[c] = _build_graph(c)
    return _graph_cache[c]


def kernel(x, gate_w, W1, b1, W2, b2):
    global LAST_RESULTS
    xt2 = np.ascontiguousarray(x.reshape(-1, D_MODEL)).astype(np.float32)
    n = xt2.shape[0]

    # --- host router (tiny: [N,1024]@[1024,8]) ---
    logits = xt2 @ gate_w.astype(np.float32).T
    logits -= logits.max(axis=-1, keepdims=True)
    probs = np.exp(logits)
    probs /= probs.sum(axis=-1, keepdims=True)
    top2 = np.argsort(-probs, axis=-1, kind="stable")[:, :TOP_K]
    wt = np.take_along_axis(probs, top2, axis=-1)
    wt = wt / (wt.sum(axis=-1, keepdims=True) + 1e-9)

    # --- dispatch: sort (token, expert) pairs by expert ---
    flat_e = top2.ravel()
    flat_t = np.repeat(np.arange(n), TOP_K)
    flat_w = wt.ravel()
    order = np.argsort(flat_e, kind="stable")
    e_sorted = flat_e[order]
    t_sorted = flat_t[order]
    w_sorted = flat_w[order]
    counts = np.bincount(e_sorted, minlength=N_EXPERTS)
    starts = np.zeros(N_EXPERTS + 1, dtype=np.int64)
    starts[1:] = np.cumsum(counts)

    c = max(CAPACITY, int(-(-counts.max() // 128)) * 128)
    # slot of each sorted pair in the concatenated [E*C] dispatch buffer,
    # then invert to per-token pair slots for the final combine
    slot = np.arange(TOP_K * n) - starts[e_sorted] + e_sorted * c
    pair_slot = np.empty(TOP_K * n, dtype=np.int64)
    pair_slot[order] = slot
    pair_slot = pair_slot.reshape(n, TOP_K)

    w1f = np.asarray(W1, dtype=np.float32)
    w2f = np.asarray(W2, dtype=np.float32)
    b1f = np.asarray(b1, dtype=np.float32)
    b2f = np.asarray(b2, dtype=np.float32)

    in_maps = []
    for e in range(N_EXPERTS):
        sel = t_sorted[starts[e]:starts[e + 1]]
        ne = len(sel)
        xe = np.zeros((D_MODEL, c), dtype=_BF)
        xe[:, :ne] = xt2[sel].T.astype(_BF)
        se = np.zeros(c, dtype=np.float32)
        se[:ne] = w_sorted[starts[e]:starts[e + 1]]
        se = np.ascontiguousarray(se.reshape(c // 128, 128).T)
        in_maps.append({
            "xt": xe,
            "w1t": np.ascontiguousarray(w1f[e].T).astype(_BF),
            "w2t": np.ascontiguousarray(w2f[e].T).astype(_BF),
            "b1": np.ascontiguousarray(b1f[e].reshape(D_FF // 128, 128).T),
            "b2bc": np.ascontiguousarray(
                np.broadcast_to(b2f[e], (128, D_MODEL))),
            "s": se,
        })

    nc = _get_graph(c)
    res = None
    for attempt in range(4):
        try:
            res = run_bass_kernel_spmd(nc, in_maps,
                                       core_ids=list(range(N_CORES)),
                                       trace=TRACE and attempt < 3)
            break
        except Exception:
            # Transient device failures (NRT_EXEC_UNIT_UNRECOVERABLE, axon
            # profile-start) clear after the terminal resets; back off and
            # retry, dropping the profiling request on the last attempt.
            if attempt == 3:
                raise
            time.sleep(20 * (attempt + 1))
    LAST_RESULTS = res

    y_all = np.concatenate([res.results[e]["out"] for e in range(N_EXPERTS)],
                           axis=0)
    combined = y_all[pair_slot[:, 0]] + y_all[pair_slot[:, 1]]
    return combined.reshape(x.shape).astype(np.float32)
